# revision 1
# baseline (speedup 1.0000x reference)
"""Trainium2 Bass kernel for an MoE layer (8 experts, top-2 routing, SwiGLU
experts + dense shared expert).

Strategy (expert-parallel over 8 NeuronCores):
  - Router (gate matmul + softmax + top-k + combine weights) and the aux/z
    losses are computed on host with jax-on-CPU, replicating the reference
    math op-for-op so the token->expert assignment matches bit-exactly.
  - Each core c is assigned expert c: the tokens routed to expert c are
    gathered on host, padded to a fixed capacity, transposed to
    feature-major layout, and shipped to core c together with that expert's
    weights. The core computes the full SwiGLU (two up-projections + silu *
    mul + down-projection) with float32r matmuls (full-rate fp32 on the
    PE array).
  - The shared expert is data-parallel: core c also computes the shared
    SwiGLU for tokens [512c, 512(c+1)).
  - Host applies the top-2 combine weights while scattering the expert
    outputs back, and adds the shared output.
"""

import numpy as np

D = 2048          # model dim
I = 1024          # expert intermediate dim
E = 8             # experts == cores
TOPK = 2
NTOK = 4096       # B*T
CE = 1152         # per-expert token capacity (3 x 384), actual max ~1063
CS = NTOK // 8    # shared-expert tokens per core
KD = D // 128     # 16 contraction tiles over D
KI = I // 128     # 8 contraction tiles over I
MI = I // 128     # 8 stage-1 output tiles
MD = D // 128     # 16 stage-2 output tiles
NE_T = 384        # moving-dim tile, expert phase (CE = 3*384)
NS_T = 512        # moving-dim tile, shared phase

AUX_COEFF = 0.01
Z_COEFF = 0.001

_PROGRAM = None


def _emit(tc, mybir, aps):
    nc = tc.nc
    f32r, f32 = mybir.dt.float32r, mybir.dt.float32
    SILU = mybir.ActivationFunctionType.Silu

    xe_r = aps["xeT"].rearrange("(ko p) c -> p ko c", p=128)    # [128,16,CE]
    xs_r = aps["xsT"].rearrange("(ko p) c -> p ko c", p=128)    # [128,16,CS]
    w1_r = aps["w1T"].rearrange("(ko p) i -> p ko i", p=128)    # [128,16,I]
    w3_r = aps["w3T"].rearrange("(ko p) i -> p ko i", p=128)
    w2_r = aps["w2T"].rearrange("(ko p) d -> p ko d", p=128)    # [128,8,D]
    sw1_r = aps["sw1T"].rearrange("(ko p) i -> p ko i", p=128)
    sw3_r = aps["sw3T"].rearrange("(ko p) i -> p ko i", p=128)
    sw2_r = aps["sw2T"].rearrange("(ko p) d -> p ko d", p=128)
    eo_r = aps["eoT"].rearrange("(mo p) c -> mo p c", p=128)    # [16,128,CE]
    sh_r = aps["shT"].rearrange("(mo p) c -> mo p c", p=128)

    with (tc.tile_pool(name="pw", bufs=4) as pw,
          tc.tile_pool(name="pt", bufs=3) as pt,
          tc.tile_pool(name="pout", bufs=3) as pout,
          tc.tile_pool(name="ps", bufs=8, space="PSUM") as ps):
        with tc.tile_pool(name="ph", bufs=1) as ph:
            h = ph.tile([128, MI, CE], f32r, name="h")
            with tc.tile_pool(name="px", bufs=1) as px:
                xe = px.tile([128, KD, CE], f32r, name="xe")
                for q in range(4):
                    nc.sync.dma_start(xe[:, q * 4:(q + 1) * 4, :],
                                      xe_r[:, q * 4:(q + 1) * 4, :])
                # ---- Expert stage 1: h = silu(w1 @ xe) * (w3 @ xe) ----
                for m in range(MI):
                    w1c = pw.tile([128, KD, 128], f32r, tag="wc", name="w1c")
                    nc.sync.dma_start(w1c[:], w1_r[:, :, m * 128:(m + 1) * 128])
                    w3c = pw.tile([128, KD, 128], f32r, tag="wc", name="w3c")
                    nc.sync.dma_start(w3c[:], w3_r[:, :, m * 128:(m + 1) * 128])
                    for n in range(CE // NE_T):
                        nsl = slice(n * NE_T, (n + 1) * NE_T)
                        pg = ps.tile([128, NE_T], f32, tag="ps", name="pg")
                        for ko in range(KD):
                            nc.tensor.matmul(pg[:], w1c[:, ko, :], xe[:, ko, nsl],
                                             start=(ko == 0), stop=(ko == KD - 1))
                        pu = ps.tile([128, NE_T], f32, tag="ps", name="pu")
                        for ko in range(KD):
                            nc.tensor.matmul(pu[:], w3c[:, ko, :], xe[:, ko, nsl],
                                             start=(ko == 0), stop=(ko == KD - 1))
                        sg = pt.tile([128, NE_T], f32, tag="sg", name="sg")
                        nc.scalar.activation(sg[:], pg[:], SILU)
                        nc.vector.tensor_mul(h[:, m, nsl], sg[:], pu[:])
            # px closed: xe space reusable
            with (tc.tile_pool(name="pxs", bufs=1) as pxs,
                  tc.tile_pool(name="phs", bufs=1) as phs):
                xs = pxs.tile([128, KD, CS], f32r, name="xs")
                for q in range(2):
                    nc.sync.dma_start(xs[:, q * 8:(q + 1) * 8, :],
                                      xs_r[:, q * 8:(q + 1) * 8, :])
                hs = phs.tile([128, KI, CS], f32r, name="hs")
                # ---- Expert stage 2: eoT = w2 @ h ----
                for m in range(MD):
                    w2c = pw.tile([128, KI, 128], f32r, tag="wc", name="w2c")
                    nc.sync.dma_start(w2c[:], w2_r[:, :, m * 128:(m + 1) * 128])
                    ot = pout.tile([128, CE], f32, tag="ot", name="ot")
                    for n in range(CE // NE_T):
                        nsl = slice(n * NE_T, (n + 1) * NE_T)
                        po = ps.tile([128, NE_T], f32, tag="ps", name="po")
                        for ko in range(KI):
                            nc.tensor.matmul(po[:], w2c[:, ko, :], h[:, ko, nsl],
                                             start=(ko == 0), stop=(ko == KI - 1))
                        nc.vector.tensor_copy(ot[:, nsl], po[:])
                    nc.sync.dma_start(eo_r[m], ot[:])
                # ---- Shared stage 1 ----
                for m in range(MI):
                    s1c = pw.tile([128, KD, 128], f32r, tag="wc", name="s1c")
                    nc.sync.dma_start(s1c[:], sw1_r[:, :, m * 128:(m + 1) * 128])
                    s3c = pw.tile([128, KD, 128], f32r, tag="wc", name="s3c")
                    nc.sync.dma_start(s3c[:], sw3_r[:, :, m * 128:(m + 1) * 128])
                    pg = ps.tile([128, NS_T], f32, tag="ps", name="spg")
                    for ko in range(KD):
                        nc.tensor.matmul(pg[:], s1c[:, ko, :], xs[:, ko, :],
                                         start=(ko == 0), stop=(ko == KD - 1))
                    pu = ps.tile([128, NS_T], f32, tag="ps", name="spu")
                    for ko in range(KD):
                        nc.tensor.matmul(pu[:], s3c[:, ko, :], xs[:, ko, :],
                                         start=(ko == 0), stop=(ko == KD - 1))
                    sg = pt.tile([128, NS_T], f32, tag="sg", name="ssg")
                    nc.scalar.activation(sg[:], pg[:], SILU)
                    nc.vector.tensor_mul(hs[:, m, :], sg[:], pu[:])
                # ---- Shared stage 2 ----
                for m in range(MD):
                    s2c = pw.tile([128, KI, 128], f32r, tag="wc", name="s2c")
                    nc.sync.dma_start(s2c[:], sw2_r[:, :, m * 128:(m + 1) * 128])
                    ot = pout.tile([128, CS], f32, tag="ot", name="sot")
                    po = ps.tile([128, NS_T], f32, tag="ps", name="spo")
                    for ko in range(KI):
                        nc.tensor.matmul(po[:], s2c[:, ko, :], hs[:, ko, :],
                                         start=(ko == 0), stop=(ko == KI - 1))
                    nc.vector.tensor_copy(ot[:], po[:])
                    nc.sync.dma_start(sh_r[m], ot[:])


def _build_program():
    import concourse.tile as tile
    from concourse import bacc, mybir

    f32r, f32 = mybir.dt.float32r, mybir.dt.float32
    nc = bacc.Bacc("TRN2", target_bir_lowering=False, debug=False, num_devices=E)
    aps = {}
    for name, shape, dt, kind in [
        ("xeT", [D, CE], f32r, "ExternalInput"),
        ("xsT", [D, CS], f32r, "ExternalInput"),
        ("w1T", [D, I], f32r, "ExternalInput"),
        ("w3T", [D, I], f32r, "ExternalInput"),
        ("w2T", [I, D], f32r, "ExternalInput"),
        ("sw1T", [D, I], f32r, "ExternalInput"),
        ("sw3T", [D, I], f32r, "ExternalInput"),
        ("sw2T", [I, D], f32r, "ExternalInput"),
        ("eoT", [D, CE], f32, "ExternalOutput"),
        ("shT", [D, CS], f32, "ExternalOutput"),
    ]:
        aps[name] = nc.dram_tensor(name, shape, dt, kind=kind).ap()

    with tile.TileContext(nc) as tc:
        _emit(tc, mybir, aps)
    nc.compile()
    return nc


def _router_host(xf, gate_w):
    """Replicate the reference router + losses with jax on CPU (bit-exact
    wrt the reference's fp32 op sequence)."""
    import jax
    import jax.numpy as jnp

    cpu = jax.devices("cpu")[0]
    with jax.default_device(cpu):
        xf_j = jnp.asarray(xf)
        gate_logits = xf_j @ jnp.asarray(gate_w).T
        scores = jax.nn.softmax(gate_logits, axis=-1)
        top_scores, top_idx = jax.lax.top_k(scores, TOPK)
        top_scores = top_scores / jnp.sum(top_scores, axis=-1, keepdims=True)
        one_hot = jax.nn.one_hot(top_idx, E, dtype=xf_j.dtype)
        combine = jnp.sum(one_hot * top_scores[..., None], axis=1)
        expert_mask = jnp.sum(one_hot, axis=1)
        f = jnp.mean(expert_mask, axis=0)
        p = jnp.mean(scores, axis=0)
        aux_loss = AUX_COEFF * E * jnp.sum(f * p)
        z = jax.nn.logsumexp(gate_logits.astype(jnp.float32), axis=-1)
        z_loss = Z_COEFF * jnp.mean(z ** 2)
        total_loss = aux_loss + z_loss
    return (np.asarray(top_idx), np.asarray(combine),
            np.asarray(total_loss, dtype=np.float32))


def _silu_np(x):
    return x / (1.0 + np.exp(-x))


def kernel(x, gate_w, w1, w3, w2, sw1, sw3, sw2):
    global _PROGRAM
    from concourse.bass_utils import run_bass_kernel_spmd

    x = np.ascontiguousarray(x, dtype=np.float32)
    gate_w = np.ascontiguousarray(gate_w, dtype=np.float32)
    w1 = np.ascontiguousarray(w1, dtype=np.float32)
    w3 = np.ascontiguousarray(w3, dtype=np.float32)
    w2 = np.ascontiguousarray(w2, dtype=np.float32)
    sw1 = np.ascontiguousarray(sw1, dtype=np.float32)
    sw3 = np.ascontiguousarray(sw3, dtype=np.float32)
    sw2 = np.ascontiguousarray(sw2, dtype=np.float32)

    B, T, Dm = x.shape
    xf = x.reshape(B * T, Dm)

    top_idx, combine, total_loss = _router_host(xf, gate_w)

    # Token dispatch; anything beyond capacity falls back to exact host math.
    idxs = []
    overflow = []
    for e in range(E):
        idx = np.nonzero((top_idx == e).any(axis=1))[0]
        if len(idx) > CE:
            overflow.extend((int(t), e) for t in idx[CE:])
            idx = idx[:CE]
        idxs.append(idx)

    sw1T = np.ascontiguousarray(sw1.T)
    sw3T = np.ascontiguousarray(sw3.T)
    sw2T = np.ascontiguousarray(sw2.T)
    in_maps = []
    for c in range(E):
        idx = idxs[c]
        xeT = np.zeros((D, CE), np.float32)
        xeT[:, :len(idx)] = xf[idx].T
        in_maps.append({
            "xeT": xeT,
            "xsT": np.ascontiguousarray(xf[c * CS:(c + 1) * CS].T),
            "w1T": np.ascontiguousarray(w1[c].T),
            "w3T": np.ascontiguousarray(w3[c].T),
            "w2T": np.ascontiguousarray(w2[c].T),
            "sw1T": sw1T,
            "sw3T": sw3T,
            "sw2T": sw2T,
        })

    if _PROGRAM is None:
        _PROGRAM = _build_program()
    res = run_bass_kernel_spmd(_PROGRAM, in_maps, core_ids=list(range(E)))

    out = np.empty((B * T, Dm), np.float32)
    for c in range(E):
        out[c * CS:(c + 1) * CS] = res.results[c]["shT"].T
    for e in range(E):
        idx = idxs[e]
        eo = res.results[e]["eoT"][:, :len(idx)].T
        out[idx] += combine[idx, e][:, None] * eo
    for t, e in overflow:
        g = w1[e] @ xf[t]
        u = w3[e] @ xf[t]
        out[t] += combine[t, e] * (w2[e] @ (_silu_np(g) * u))

    return out.reshape(B, T, Dm), total_loss


# revision 2
# speedup vs baseline: 1.1833x; 1.1833x over previous
"""Trainium2 Bass kernel for an MoE layer (8 experts, top-2 routing, SwiGLU
experts + dense shared expert).

Strategy (expert-parallel over 8 NeuronCores):
  - Router (gate matmul + softmax + top-k + combine weights) and the aux/z
    losses are computed on host with jax-on-CPU, replicating the reference
    math op-for-op so the token->expert assignment matches bit-exactly.
  - Each core c is assigned expert c: the tokens routed to expert c are
    gathered on host, padded to a fixed capacity CE=1024, transposed to
    feature-major layout, and shipped to core c together with that expert's
    weights (pre-blocked on host so every weight DMA is a contiguous
    8KB-per-partition read). The core computes the full SwiGLU with
    float32r matmuls. Tokens beyond capacity (rare, ~90 pairs for the
    fixed seed) are computed exactly on host.
  - The shared expert is data-parallel: core c computes the shared SwiGLU
    for tokens [512c, 512(c+1)).
  - Host applies the top-2 combine weights while scattering expert outputs
    back, and adds the shared output.
"""

import numpy as np

D = 2048          # model dim
I = 1024          # expert intermediate dim
E = 8             # experts == cores
TOPK = 2
NTOK = 4096       # B*T
CE = 1024         # per-expert token capacity (2 x 512)
CS = NTOK // 8    # shared-expert tokens per core
KD = D // 128     # 16 contraction tiles over D
KI = I // 128     # 8 contraction tiles over I
MI = I // 128     # 8 stage-1 output tiles
MD = D // 128     # 16 stage-2 output tiles
NE_T = 512        # moving-dim tile, expert phase (CE = 2*512)
NS_T = 512        # moving-dim tile, shared phase

AUX_COEFF = 0.01
Z_COEFF = 0.001

_PROGRAM = None


def _emit(tc, mybir, aps):
    nc = tc.nc
    f32r, f32 = mybir.dt.float32r, mybir.dt.float32
    SILU = mybir.ActivationFunctionType.Silu
    NE = CE // NE_T   # 2 moving tiles in expert phase

    xe_r = aps["xeT"].rearrange("(ko p) c -> p ko c", p=128)    # [128,16,CE]
    xs_r = aps["xsT"].rearrange("(ko p) c -> p ko c", p=128)    # [128,16,CS]
    eo_r = aps["eoT"].rearrange("(mo p) c -> mo p c", p=128)    # [16,128,CE]
    sh_r = aps["shT"].rearrange("(mo p) c -> mo p c", p=128)
    w1_b, w3_b, w2_b = aps["w1B"], aps["w3B"], aps["w2B"]       # [m,128,ko,128]
    sw1_b, sw3_b, sw2_b = aps["sw1B"], aps["sw3B"], aps["sw2B"]

    with (tc.tile_pool(name="pw", bufs=4) as pw,
          tc.tile_pool(name="pt", bufs=4) as pt,
          tc.tile_pool(name="pout", bufs=3) as pout,
          tc.tile_pool(name="ps", bufs=8, space="PSUM") as ps):
        with tc.tile_pool(name="ph", bufs=1) as ph:
            h = ph.tile([128, MI, CE], f32r, name="h")
            with tc.tile_pool(name="px", bufs=1) as px:
                xe = px.tile([128, KD, CE], f32r, name="xe")
                for q in range(4):
                    nc.scalar.dma_start(xe[:, q * 4:(q + 1) * 4, :],
                                        xe_r[:, q * 4:(q + 1) * 4, :])
                # ---- Expert stage 1: h = silu(w1 @ xe) * (w3 @ xe) ----
                for m in range(MI):
                    w1c = pw.tile([128, KD, 128], f32r, tag="wc", name="w1c")
                    nc.sync.dma_start(w1c[:], w1_b[m])
                    w3c = pw.tile([128, KD, 128], f32r, tag="wc", name="w3c")
                    nc.sync.dma_start(w3c[:], w3_b[m])
                    pgs = [ps.tile([128, NE_T], f32, tag="ps", name=f"pg{n}")
                           for n in range(NE)]
                    for ko in range(KD):
                        for n in range(NE):
                            nsl = slice(n * NE_T, (n + 1) * NE_T)
                            nc.tensor.matmul(pgs[n][:], w1c[:, ko, :],
                                             xe[:, ko, nsl],
                                             start=(ko == 0), stop=(ko == KD - 1))
                    pus = [ps.tile([128, NE_T], f32, tag="ps", name=f"pu{n}")
                           for n in range(NE)]
                    for ko in range(KD):
                        for n in range(NE):
                            nsl = slice(n * NE_T, (n + 1) * NE_T)
                            nc.tensor.matmul(pus[n][:], w3c[:, ko, :],
                                             xe[:, ko, nsl],
                                             start=(ko == 0), stop=(ko == KD - 1))
                    for n in range(NE):
                        nsl = slice(n * NE_T, (n + 1) * NE_T)
                        sg = pt.tile([128, NE_T], f32, tag="sg", name="sg")
                        nc.scalar.activation(sg[:], pgs[n][:], SILU)
                        nc.vector.tensor_mul(h[:, m, nsl], sg[:], pus[n][:])
            # px closed: xe space reusable
            with (tc.tile_pool(name="pxs", bufs=1) as pxs,
                  tc.tile_pool(name="phs", bufs=1) as phs):
                xs = pxs.tile([128, KD, CS], f32r, name="xs")
                for q in range(2):
                    nc.scalar.dma_start(xs[:, q * 8:(q + 1) * 8, :],
                                        xs_r[:, q * 8:(q + 1) * 8, :])
                hs = phs.tile([128, KI, CS], f32r, name="hs")
                # ---- Expert stage 2: eoT = w2 @ h ----
                for m in range(MD):
                    w2c = pw.tile([128, KI, 128], f32r, tag="wc", name="w2c")
                    nc.sync.dma_start(w2c[:], w2_b[m])
                    ot = pout.tile([128, CE], f32, tag="ot", name="ot")
                    pos = [ps.tile([128, NE_T], f32, tag="ps", name=f"po{n}")
                           for n in range(NE)]
                    for ko in range(KI):
                        for n in range(NE):
                            nsl = slice(n * NE_T, (n + 1) * NE_T)
                            nc.tensor.matmul(pos[n][:], w2c[:, ko, :],
                                             h[:, ko, nsl],
                                             start=(ko == 0), stop=(ko == KI - 1))
                    for n in range(NE):
                        nsl = slice(n * NE_T, (n + 1) * NE_T)
                        nc.vector.tensor_copy(ot[:, nsl], pos[n][:])
                    nc.scalar.dma_start(eo_r[m], ot[:])
                # ---- Shared stage 1 ----
                for m in range(MI):
                    s1c = pw.tile([128, KD, 128], f32r, tag="wc", name="s1c")
                    nc.sync.dma_start(s1c[:], sw1_b[m])
                    s3c = pw.tile([128, KD, 128], f32r, tag="wc", name="s3c")
                    nc.sync.dma_start(s3c[:], sw3_b[m])
                    pg = ps.tile([128, NS_T], f32, tag="ps", name="spg")
                    for ko in range(KD):
                        nc.tensor.matmul(pg[:], s1c[:, ko, :], xs[:, ko, :],
                                         start=(ko == 0), stop=(ko == KD - 1))
                    pu = ps.tile([128, NS_T], f32, tag="ps", name="spu")
                    for ko in range(KD):
                        nc.tensor.matmul(pu[:], s3c[:, ko, :], xs[:, ko, :],
                                         start=(ko == 0), stop=(ko == KD - 1))
                    sg = pt.tile([128, NS_T], f32, tag="sg", name="ssg")
                    nc.scalar.activation(sg[:], pg[:], SILU)
                    nc.vector.tensor_mul(hs[:, m, :], sg[:], pu[:])
                # ---- Shared stage 2 ----
                for m in range(MD):
                    s2c = pw.tile([128, KI, 128], f32r, tag="wc", name="s2c")
                    nc.sync.dma_start(s2c[:], sw2_b[m])
                    ot = pout.tile([128, CS], f32, tag="ot", name="sot")
                    po = ps.tile([128, NS_T], f32, tag="ps", name="spo")
                    for ko in range(KI):
                        nc.tensor.matmul(po[:], s2c[:, ko, :], hs[:, ko, :],
                                         start=(ko == 0), stop=(ko == KI - 1))
                    nc.vector.tensor_copy(ot[:], po[:])
                    nc.scalar.dma_start(sh_r[m], ot[:])


def _build_program():
    import concourse.tile as tile
    from concourse import bacc, mybir

    f32r, f32 = mybir.dt.float32r, mybir.dt.float32
    nc = bacc.Bacc("TRN2", target_bir_lowering=False, debug=False, num_devices=E)
    aps = {}
    for name, shape, dt, kind in [
        ("xeT", [D, CE], f32r, "ExternalInput"),
        ("xsT", [D, CS], f32r, "ExternalInput"),
        ("w1B", [MI, 128, KD, 128], f32r, "ExternalInput"),
        ("w3B", [MI, 128, KD, 128], f32r, "ExternalInput"),
        ("w2B", [MD, 128, KI, 128], f32r, "ExternalInput"),
        ("sw1B", [MI, 128, KD, 128], f32r, "ExternalInput"),
        ("sw3B", [MI, 128, KD, 128], f32r, "ExternalInput"),
        ("sw2B", [MD, 128, KI, 128], f32r, "ExternalInput"),
        ("eoT", [D, CE], f32, "ExternalOutput"),
        ("shT", [D, CS], f32, "ExternalOutput"),
    ]:
        aps[name] = nc.dram_tensor(name, shape, dt, kind=kind).ap()

    with tile.TileContext(nc) as tc:
        _emit(tc, mybir, aps)
    nc.compile()
    return nc


def _block_up(wT):
    """[D(=ko*128+p), I(=m*128+i)] -> [m, p, ko, i] contiguous blocks."""
    return np.ascontiguousarray(
        wT.reshape(KD, 128, MI, 128).transpose(2, 1, 0, 3))


def _block_down(wT):
    """[I(=ko*128+p), D(=m*128+d)] -> [m, p, ko, d] contiguous blocks."""
    return np.ascontiguousarray(
        wT.reshape(KI, 128, MD, 128).transpose(2, 1, 0, 3))


def _router_host(xf, gate_w):
    """Replicate the reference router + losses with jax on CPU (bit-exact
    wrt the reference's fp32 op sequence)."""
    import jax
    import jax.numpy as jnp

    cpu = jax.devices("cpu")[0]
    with jax.default_device(cpu):
        xf_j = jnp.asarray(xf)
        gate_logits = xf_j @ jnp.asarray(gate_w).T
        scores = jax.nn.softmax(gate_logits, axis=-1)
        top_scores, top_idx = jax.lax.top_k(scores, TOPK)
        top_scores = top_scores / jnp.sum(top_scores, axis=-1, keepdims=True)
        one_hot = jax.nn.one_hot(top_idx, E, dtype=xf_j.dtype)
        combine = jnp.sum(one_hot * top_scores[..., None], axis=1)
        expert_mask = jnp.sum(one_hot, axis=1)
        f = jnp.mean(expert_mask, axis=0)
        p = jnp.mean(scores, axis=0)
        aux_loss = AUX_COEFF * E * jnp.sum(f * p)
        z = jax.nn.logsumexp(gate_logits.astype(jnp.float32), axis=-1)
        z_loss = Z_COEFF * jnp.mean(z ** 2)
        total_loss = aux_loss + z_loss
    return (np.asarray(top_idx), np.asarray(combine),
            np.asarray(total_loss, dtype=np.float32))


def _silu_np(x):
    return x / (1.0 + np.exp(-x))


def kernel(x, gate_w, w1, w3, w2, sw1, sw3, sw2):
    global _PROGRAM
    from concourse.bass_utils import run_bass_kernel_spmd

    x = np.ascontiguousarray(x, dtype=np.float32)
    gate_w = np.ascontiguousarray(gate_w, dtype=np.float32)
    w1 = np.ascontiguousarray(w1, dtype=np.float32)
    w3 = np.ascontiguousarray(w3, dtype=np.float32)
    w2 = np.ascontiguousarray(w2, dtype=np.float32)
    sw1 = np.ascontiguousarray(sw1, dtype=np.float32)
    sw3 = np.ascontiguousarray(sw3, dtype=np.float32)
    sw2 = np.ascontiguousarray(sw2, dtype=np.float32)

    B, T, Dm = x.shape
    xf = x.reshape(B * T, Dm)

    top_idx, combine, total_loss = _router_host(xf, gate_w)

    # Token dispatch; anything beyond capacity falls back to exact host math.
    idxs = []
    overflow = {}
    for e in range(E):
        idx = np.nonzero((top_idx == e).any(axis=1))[0]
        if len(idx) > CE:
            overflow[e] = idx[CE:]
            idx = idx[:CE]
        idxs.append(idx)

    sw1B = _block_up(sw1.T)
    sw3B = _block_up(sw3.T)
    sw2B = _block_down(sw2.T)
    in_maps = []
    for c in range(E):
        idx = idxs[c]
        xeT = np.zeros((D, CE), np.float32)
        xeT[:, :len(idx)] = xf[idx].T
        in_maps.append({
            "xeT": xeT,
            "xsT": np.ascontiguousarray(xf[c * CS:(c + 1) * CS].T),
            "w1B": _block_up(w1[c].T),
            "w3B": _block_up(w3[c].T),
            "w2B": _block_down(w2[c].T),
            "sw1B": sw1B,
            "sw3B": sw3B,
            "sw2B": sw2B,
        })

    if _PROGRAM is None:
        _PROGRAM = _build_program()
    res = run_bass_kernel_spmd(_PROGRAM, in_maps, core_ids=list(range(E)))

    out = np.empty((B * T, Dm), np.float32)
    for c in range(E):
        out[c * CS:(c + 1) * CS] = res.results[c]["shT"].T
    for e in range(E):
        idx = idxs[e]
        eo = res.results[e]["eoT"][:, :len(idx)].T
        out[idx] += combine[idx, e][:, None] * eo
    for e, extra in overflow.items():
        xo = xf[extra]                       # [n, D]
        g = xo @ w1[e].T
        u = xo @ w3[e].T
        eo = (_silu_np(g) * u) @ w2[e].T
        out[extra] += combine[extra, e][:, None] * eo

    return out.reshape(B, T, Dm), total_loss
